# revision 8
# baseline (speedup 1.0000x reference)
"""Trainium2 Bass kernel for nn_AssembleAttentionBlock.

The reference module computes, per (batch b, ref r):
    kv[b,r,:]  = (instance_tokens[b,r] @ W_layout.T + b_layout) * (1 - alpha[b,r])
    scores     = q . kv   (per head)  -> softmax over a SINGLE key axis == 1.0
    attn       = kv (broadcast over the ref's T tokens, independent of q)
    out[b,r,:] = kv[b,r,:] @ W_out.T
    result     = image_tokens with rows img_idxs[r, :] replaced by out[b,r,:]

Since the softmax is over an axis of size 1, the whole query path
(gather + W_q projection + scores) cancels out, and each of the T tokens
of ref r receives the same row out[b,r,:].  The device kernel therefore
computes the two (16 x 3072) @ (3072 x 3072) projections, tensor-parallel
over the 3072 hidden dim across 8 NeuronCores:

  core c:  kv_c = scaled/biased A @ W_layout.T columns [c*384, (c+1)*384)
           partial_c = kv_c @ W_out.T rows [c*384, (c+1)*384)   (16 x 3072)
  host:    out16 = sum_c partial_c ; broadcast + scatter into image_tokens.

The bias row and the (1-alpha) scaling are folded into the matmul by
augmenting the contraction dim with one extra row (K = 3072+1 -> 3200 padded).
Weights are pre-arranged on host into the exact SBUF tile layouts so every
DMA is a fully contiguous block.  Matmuls keep the stationary operand tiny
(16 cols) and the moving operand large (384/512 cols); kv is transposed
on the PE (via a 16x16 identity) to feed stage 2's contraction.
"""

import os

import numpy as np

DIM = 3072
B = 2
R = 8
M = B * R            # 16 rows through both projections
N_IMG = 4096
N_CORES = 8
SH = DIM // N_CORES  # 384 hidden columns per core
NKC = 25             # contraction chunks of 128 (3072 + 1 bias row, zero-padded)
KP = NKC * 128       # 3200
NT = SH // 128       # 3 row-tiles of kv^T per core
NB = DIM // 512      # 6 output column banks
W1SPLIT = (13, 12)   # wt1 DMA'd in 2 chunks of k-blocks (~2.5MB each)
W2SPLIT = (3, 3)     # wt2 DMA'd in 2 chunks of output banks (~2.36MB each)
NKA = NKC + 1        # ats chunks + identity chunk

MM_DTYPE = os.environ.get("BASS_MM_DTYPE", "float32r")

_cache = {}
_last_in_maps = None


def _np_in_dtype():
    if MM_DTYPE == "bfloat16":
        import ml_dtypes
        return ml_dtypes.bfloat16
    return np.float32


def _build():
    """Build + compile the per-core Bass program (same NEFF on all cores).

    Raw-Bass (no TileContext): a small static program with hand-placed
    semaphores, to avoid Tile's ~300 event-semaphore instructions, its
    instruction-fetch startup cost and its end-of-kernel drain+barrier tail.

    Semaphore protocol:
      dmaI: +16 per input DMA, issue order ats, ident, wt1[0..4], wt2[0..2]
      pe:   1 = stage-1 accumulation done; 2 = transposes done;
            3+nb = stage-2 bank nb matmuls done
      dve:  1 = kv copied to SBUF; 2 = kv^T tiles copied;
            3+nb = output bank nb copied to SBUF
      dmaO: +16 per output DMA (6 total, issued on the ACT HWDGE ring so
            they never queue behind input DMAs on the SP ring)
    """
    from contextlib import ExitStack

    import concourse.bacc as bacc
    import concourse.mybir as mybir

    nc = bacc.Bacc("TRN2", target_bir_lowering=False, debug=False,
                   num_devices=N_CORES)
    DT = {
        "float32": mybir.dt.float32,
        "float32r": mybir.dt.float32r,
        "bfloat16": mybir.dt.bfloat16,
    }[MM_DTYPE]
    F32 = mybir.dt.float32

    wt1 = nc.dram_tensor("wt1", [128, NKC, SH], DT, kind="ExternalInput")
    wt2 = nc.dram_tensor("wt2", [128, NB, NT, 512], DT, kind="ExternalInput")
    ats = nc.dram_tensor("ats", [128, NKA, M], DT, kind="ExternalInput")
    outp = nc.dram_tensor("out_partial", [M, DIM], F32, kind="ExternalOutput")

    with ExitStack() as ctx:
        ats_sb = ctx.enter_context(nc.sbuf_tensor("ats_sb", [128, NKA, M], DT))
        wt1_sb = ctx.enter_context(nc.sbuf_tensor("wt1_sb", [128, NKC, SH], DT))
        wt2_sb = ctx.enter_context(
            nc.sbuf_tensor("wt2_sb", [128, NB, NT, 512], DT))
        kv_sb = ctx.enter_context(nc.sbuf_tensor("kv_sb", [M, SH], DT))
        kvT_sb = [ctx.enter_context(nc.sbuf_tensor(f"kvT_sb{t}", [128, M], DT))
                  for t in range(NT)]
        o_sb = [ctx.enter_context(nc.sbuf_tensor(f"o_sb{i}", [M, 512], F32))
                for i in range(NB)]

        kv_ps = ctx.enter_context(nc.psum_tensor("kv_ps", [M, SH], F32))
        kvT_ps = [ctx.enter_context(nc.psum_tensor(f"kvT_ps{t}", [128, M], DT))
                  for t in range(NT)]
        o_ps = [ctx.enter_context(nc.psum_tensor(f"o_ps{i}", [M, 512], F32))
                for i in range(2)]

        dmaI = ctx.enter_context(nc.semaphore("dmaI"))   # ats + wt1 chunks
        dmaJ = ctx.enter_context(nc.semaphore("dmaJ"))   # wt2 chunks
        dmaO = ctx.enter_context(nc.semaphore("dmaO"))
        pe = ctx.enter_context(nc.semaphore("pe"))
        dve = ctx.enter_context(nc.semaphore("dve"))

        block = ctx.enter_context(nc.Block(no_gpsimd_drain=True))

        id_sb = ats_sb[:M, NKC, :]          # identity lives in ats chunk 25

        @block.sync
        def _(sync):
            sync.dma_start(ats_sb[:], ats[:]).then_inc(dmaI, 16)
            k0 = 0
            for n in W1SPLIT:
                sync.dma_start(
                    wt1_sb[:, k0:k0 + n, :], wt1[:, k0:k0 + n, :]
                ).then_inc(dmaI, 16)
                k0 += n

        @block.tensor
        def _(tensor):
            # stage 1: kv (16 x 384) accumulated over 25 K-chunks
            k_edges = [0, W1SPLIT[0], NKC]
            for k in range(NKC):
                if k in k_edges:
                    tensor.wait_ge(dmaI, 32 + 16 * k_edges.index(k))
                mm = tensor.matmul(
                    kv_ps[:],
                    ats_sb[:, k, :],
                    wt1_sb[:, k, :],
                    start=(k == 0),
                    stop=(k == NKC - 1),
                )
                if k == NKC - 1:
                    mm.then_inc(pe, 1)
            # transpose kv -> kv^T (3 x [128, 16])
            tensor.wait_ge(dve, 1)
            for t in range(NT):
                mm = tensor.transpose(
                    kvT_ps[t][:], kv_sb[:, 128 * t:128 * (t + 1)], id_sb)
                if t == NT - 1:
                    mm.then_inc(pe, 1)
            # stage 2: partial (16 x 3072), contraction over this core's 384
            tensor.wait_ge(dve, 2)
            nb_edges = [0, W2SPLIT[0]]
            for nb in range(NB):
                if nb in nb_edges:
                    tensor.wait_ge(dmaJ, 16 * (nb_edges.index(nb) + 1))
                if nb >= 2:
                    tensor.wait_ge(dve, 1 + nb)  # WAR: o_ps[nb%2] free again
                for j in range(NT):
                    mm = tensor.matmul(
                        o_ps[nb % 2][:],
                        kvT_sb[j][:],
                        wt2_sb[:, nb, j, :],
                        start=(j == 0),
                        stop=(j == NT - 1),
                    )
                    if j == NT - 1:
                        mm.then_inc(pe, 1)

        @block.vector
        def _(vector):
            vector.wait_ge(pe, 1)
            vector.tensor_copy(kv_sb[:], kv_ps[:]).then_inc(dve, 1)
            vector.wait_ge(pe, 2)
            for t in range(NT):
                cp = vector.tensor_copy(kvT_sb[t][:], kvT_ps[t][:])
                if t == NT - 1:
                    cp.then_inc(dve, 1)
            for nb in range(NB):
                vector.wait_ge(pe, 3 + nb)
                vector.tensor_copy(
                    o_sb[nb][:], o_ps[nb % 2][:]).then_inc(dve, 1)

        @block.scalar
        def _(scalar):
            b0 = 0
            for n in W2SPLIT:
                scalar.dma_start(
                    wt2_sb[:, b0:b0 + n, :, :], wt2[:, b0:b0 + n, :, :]
                ).then_inc(dmaJ, 16)
                b0 += n
            for nb in range(NB):
                scalar.wait_ge(dve, 3 + nb)
                scalar.dma_start(
                    outp[:, 512 * nb:512 * (nb + 1)], o_sb[nb][:]
                ).then_inc(dmaO, 16)
            scalar.wait_ge(dmaO, 16 * NB)

    nc.compile()
    return nc


def kernel(instance_tokens, image_tokens, bbox_masks, alpha,
           W_layout, b_layout, W_q, W_out, img_idxs):
    from concourse.bass_utils import run_bass_kernel_spmd

    instance_tokens = np.asarray(instance_tokens, dtype=np.float32)
    image_tokens = np.asarray(image_tokens, dtype=np.float32)
    alpha = np.asarray(alpha, dtype=np.float32)
    W_layout = np.asarray(W_layout, dtype=np.float32)
    b_layout = np.asarray(b_layout, dtype=np.float32)
    W_out = np.asarray(W_out, dtype=np.float32)
    img_idxs = np.asarray(img_idxs)

    ndt = _np_in_dtype()
    A = instance_tokens.reshape(M, DIM)                  # (16, 3072)
    s = (1.0 - alpha).reshape(M).astype(np.float32)      # (16,)

    # ats: augmented, pre-scaled A^T -> SBUF layout (128, 25, 16)
    ats_aug = np.zeros((KP, M), dtype=np.float32)
    ats_aug[:DIM, :] = A.T * s[None, :]
    ats_aug[DIM, :] = s
    ats_host = np.zeros((128, NKA, M), dtype=np.float32)
    ats_host[:, :NKC, :] = ats_aug.reshape(NKC, 128, M).transpose(1, 0, 2)
    ats_host[:M, NKC, :] = np.eye(M, dtype=np.float32)
    ats_host = np.ascontiguousarray(ats_host).astype(ndt)

    in_maps = []
    for c in range(N_CORES):
        c0, c1 = c * SH, (c + 1) * SH
        # wt1: W_layout^T columns chunk + bias row -> (128, 25, 384)
        w1t = np.zeros((KP, SH), dtype=np.float32)
        w1t[:DIM, :] = W_layout[c0:c1, :].T
        w1t[DIM, :] = b_layout[c0:c1]
        wt1_host = np.ascontiguousarray(
            w1t.reshape(NKC, 128, SH).transpose(1, 0, 2)).astype(ndt)
        # wt2: W_out^T rows chunk -> (128, 6, 3, 512)
        w2t = np.ascontiguousarray(W_out[:, c0:c1].T)    # (384, 3072)
        wt2_host = np.ascontiguousarray(
            w2t.reshape(NT, 128, NB, 512).transpose(1, 2, 0, 3)).astype(ndt)
        in_maps.append({"wt1": wt1_host, "wt2": wt2_host, "ats": ats_host})

    global _last_in_maps
    _last_in_maps = in_maps
    if "nc" not in _cache:
        _cache["nc"] = _build()
    res = run_bass_kernel_spmd(_cache["nc"], in_maps,
                               core_ids=list(range(N_CORES)))

    out16 = np.zeros((M, DIM), dtype=np.float32)
    for c in range(N_CORES):
        out16 += res.results[c]["out_partial"]

    # broadcast each ref row over its T tokens, then scatter by img_idxs
    T = img_idxs.shape[1]
    out_flat = np.repeat(
        out16.reshape(B, R, 1, DIM), T, axis=2).reshape(B, R * T, DIM)
    flat_idx = np.asarray(img_idxs, dtype=np.int64).reshape(-1)
    if R * T == N_IMG and np.array_equal(flat_idx, np.arange(N_IMG)):
        return np.ascontiguousarray(out_flat)
    result = image_tokens.copy()
    result[:, flat_idx, :] = out_flat
    return result


# revision 10
# speedup vs baseline: 1.6446x; 1.6446x over previous
"""Trainium2 Bass kernel for nn_AssembleAttentionBlock.

The reference module computes, per (batch b, ref r):
    kv[b,r,:]  = (instance_tokens[b,r] @ W_layout.T + b_layout) * (1 - alpha[b,r])
    scores     = q . kv   (per head)  -> softmax over a SINGLE key axis == 1.0
    attn       = kv (broadcast over the ref's T tokens, independent of q)
    out[b,r,:] = kv[b,r,:] @ W_out.T
    result     = image_tokens with rows img_idxs[r, :] replaced by out[b,r,:]

Since the softmax is over an axis of size 1, the whole query path
(gather + W_q projection + scores) cancels out, and each of the T tokens
of ref r receives the same row out[b,r,:].  The device kernel therefore
computes the two (16 x 3072) @ (3072 x 3072) projections, tensor-parallel
over the 3072 hidden dim across 8 NeuronCores:

  core c:  kv_c = scaled/biased A @ W_layout.T columns [c*384, (c+1)*384)
           partial_c = kv_c @ W_out.T rows [c*384, (c+1)*384)   (16 x 3072)
  host:    out16 = sum_c partial_c ; broadcast + scatter into image_tokens.

The bias row and the (1-alpha) scaling are folded into the matmul by
augmenting the contraction dim with one extra row (K = 3072+1 -> 3200 padded).
Weights are pre-arranged on host into the exact SBUF tile layouts so every
DMA is a fully contiguous block.  Matmuls keep the stationary operand tiny
(16 cols) and the moving operand large (384/512 cols); kv is transposed
on the PE (via a 16x16 identity) to feed stage 2's contraction.
"""

import os

import numpy as np

DIM = 3072
B = 2
R = 8
M = B * R            # 16 rows through both projections
N_IMG = 4096
N_CORES = 8
SH = DIM // N_CORES  # 384 hidden columns per core
NKC = 25             # contraction chunks of 128 (3072 + 1 bias row, zero-padded)
KP = NKC * 128       # 3200
NT = SH // 128       # 3 row-tiles of kv^T per core
NB = DIM // 512      # 6 output column banks
W1SPLIT = (9, 8, 8)  # wt1 DMA chunks (k-blocks)
W2SPLIT = (3, 3)     # wt2 DMA chunks (output banks)
NKA = NKC + 1        # ats chunks + identity chunk

MM_DTYPE = os.environ.get("BASS_MM_DTYPE", "float32r")

_cache = {}
_last_in_maps = None


def _np_in_dtype():
    if MM_DTYPE == "bfloat16":
        import ml_dtypes
        return ml_dtypes.bfloat16
    if MM_DTYPE == "float16":
        return np.float16
    return np.float32


def _build():
    """Build + compile the per-core Bass program (same NEFF on all cores).

    Raw-Bass (no TileContext): a small static program with hand-placed
    semaphores, to avoid Tile's ~300 event-semaphore instructions, its
    instruction-fetch startup cost and its end-of-kernel drain+barrier tail.

    Semaphore protocol:
      dmaI: +16 per input DMA, issue order ats, ident, wt1[0..4], wt2[0..2]
      pe:   1 = stage-1 accumulation done; 2 = transposes done;
            3+nb = stage-2 bank nb matmuls done
      dve:  1 = kv copied to SBUF; 2 = kv^T tiles copied;
            3+nb = output bank nb copied to SBUF
      dmaO: +16 per output DMA (6 total, issued on the ACT HWDGE ring so
            they never queue behind input DMAs on the SP ring)
    """
    from contextlib import ExitStack

    import concourse.bacc as bacc
    import concourse.mybir as mybir

    nc = bacc.Bacc("TRN2", target_bir_lowering=False, debug=False,
                   num_devices=N_CORES)
    DT = {
        "float32": mybir.dt.float32,
        "float32r": mybir.dt.float32r,
        "bfloat16": mybir.dt.bfloat16,
        "float16": mybir.dt.float16,
    }[MM_DTYPE]
    F32 = mybir.dt.float32

    wt1 = nc.dram_tensor("wt1", [128, NKC, SH], DT, kind="ExternalInput")
    wt2 = nc.dram_tensor("wt2", [128, NB, NT, 512], DT, kind="ExternalInput")
    ats = nc.dram_tensor("ats", [128, NKA, M], DT, kind="ExternalInput")
    outp = nc.dram_tensor("out_partial", [M, DIM], F32, kind="ExternalOutput")

    with ExitStack() as ctx:
        ats_sb = ctx.enter_context(nc.sbuf_tensor("ats_sb", [128, NKA, M], DT))
        wt1_sb = ctx.enter_context(nc.sbuf_tensor("wt1_sb", [128, NKC, SH], DT))
        wt2_sb = ctx.enter_context(
            nc.sbuf_tensor("wt2_sb", [128, NB, NT, 512], DT))
        kv_sb = ctx.enter_context(nc.sbuf_tensor("kv_sb", [M, SH], DT))
        kvT_sb = [ctx.enter_context(nc.sbuf_tensor(f"kvT_sb{t}", [128, M], DT))
                  for t in range(NT)]
        o_sb = [ctx.enter_context(nc.sbuf_tensor(f"o_sb{i}", [M, 512], F32))
                for i in range(NB)]

        kv_ps = ctx.enter_context(nc.psum_tensor("kv_ps", [M, SH], F32))
        kvT_ps = [ctx.enter_context(nc.psum_tensor(f"kvT_ps{t}", [128, M], DT))
                  for t in range(NT)]
        o_ps = [ctx.enter_context(nc.psum_tensor(f"o_ps{i}", [M, 512], F32))
                for i in range(2)]

        s_ats = ctx.enter_context(nc.semaphore("s_ats"))
        s_w1 = [ctx.enter_context(nc.semaphore(f"s_w1_{i}"))
                for i in range(len(W1SPLIT))]
        s_w2 = [ctx.enter_context(nc.semaphore(f"s_w2_{i}"))
                for i in range(len(W2SPLIT))]
        dmaO = ctx.enter_context(nc.semaphore("dmaO"))
        pe = ctx.enter_context(nc.semaphore("pe"))
        dve = ctx.enter_context(nc.semaphore("dve"))

        block = ctx.enter_context(nc.Block(no_gpsimd_drain=True))

        id_sb = ats_sb[:M, NKC, :]          # identity lives in ats chunk 25

        @block.sync
        def _(sync):
            sync.dma_start(ats_sb[:], ats[:]).then_inc(s_ats, 16)
            k0 = 0
            for i, n in enumerate(W1SPLIT):
                sync.dma_start(
                    wt1_sb[:, k0:k0 + n, :], wt1[:, k0:k0 + n, :]
                ).then_inc(s_w1[i], 16)
                k0 += n
            b0 = 0
            for i, n in enumerate(W2SPLIT):
                sync.dma_start(
                    wt2_sb[:, b0:b0 + n, :, :], wt2[:, b0:b0 + n, :, :]
                ).then_inc(s_w2[i], 16)
                b0 += n

        @block.tensor
        def _(tensor):
            # stage 1: kv (16 x 384) accumulated over 25 K-chunks
            k_edges = list(np.cumsum((0,) + W1SPLIT))[:-1]
            tensor.wait_ge(s_ats, 16)
            for k in range(NKC):
                if k in k_edges:
                    tensor.wait_ge(s_w1[k_edges.index(k)], 16)
                mm = tensor.matmul(
                    kv_ps[:],
                    ats_sb[:, k, :],
                    wt1_sb[:, k, :],
                    start=(k == 0),
                    stop=(k == NKC - 1),
                )
                if k == NKC - 1:
                    mm.then_inc(pe, 1)
            # transpose kv -> kv^T (3 x [128, 16])
            tensor.wait_ge(dve, 1)
            for t in range(NT):
                mm = tensor.transpose(
                    kvT_ps[t][:], kv_sb[:, 128 * t:128 * (t + 1)], id_sb)
                if t == NT - 1:
                    mm.then_inc(pe, 1)
            # stage 2: partial (16 x 3072), contraction over this core's 384
            tensor.wait_ge(dve, 2)
            nb_edges = list(np.cumsum((0,) + W2SPLIT))[:-1]
            for nb in range(NB):
                if nb in nb_edges:
                    tensor.wait_ge(s_w2[nb_edges.index(nb)], 16)
                if nb >= 2:
                    tensor.wait_ge(dve, 1 + nb)  # WAR: o_ps[nb%2] free again
                for j in range(NT):
                    mm = tensor.matmul(
                        o_ps[nb % 2][:],
                        kvT_sb[j][:],
                        wt2_sb[:, nb, j, :],
                        start=(j == 0),
                        stop=(j == NT - 1),
                    )
                    if j == NT - 1:
                        mm.then_inc(pe, 1)

        @block.vector
        def _(vector):
            vector.wait_ge(pe, 1)
            vector.tensor_copy(kv_sb[:], kv_ps[:]).then_inc(dve, 1)
            vector.wait_ge(pe, 2)
            for t in range(NT):
                cp = vector.tensor_copy(kvT_sb[t][:], kvT_ps[t][:])
                if t == NT - 1:
                    cp.then_inc(dve, 1)
            for nb in range(NB):
                vector.wait_ge(pe, 3 + nb)
                vector.tensor_copy(
                    o_sb[nb][:], o_ps[nb % 2][:]).then_inc(dve, 1)

        @block.scalar
        def _(scalar):
            for nb in range(NB):
                scalar.wait_ge(dve, 3 + nb)
                scalar.dma_start(
                    outp[:, 512 * nb:512 * (nb + 1)], o_sb[nb][:]
                ).then_inc(dmaO, 16)
            scalar.wait_ge(dmaO, 16 * NB)

    nc.compile()
    return nc


def kernel(instance_tokens, image_tokens, bbox_masks, alpha,
           W_layout, b_layout, W_q, W_out, img_idxs):
    from concourse.bass_utils import run_bass_kernel_spmd

    instance_tokens = np.asarray(instance_tokens, dtype=np.float32)
    image_tokens = np.asarray(image_tokens, dtype=np.float32)
    alpha = np.asarray(alpha, dtype=np.float32)
    W_layout = np.asarray(W_layout, dtype=np.float32)
    b_layout = np.asarray(b_layout, dtype=np.float32)
    W_out = np.asarray(W_out, dtype=np.float32)
    img_idxs = np.asarray(img_idxs)

    ndt = _np_in_dtype()
    A = instance_tokens.reshape(M, DIM)                  # (16, 3072)
    s = (1.0 - alpha).reshape(M).astype(np.float32)      # (16,)

    # ats: augmented, pre-scaled A^T -> SBUF layout (128, 25, 16)
    ats_aug = np.zeros((KP, M), dtype=np.float32)
    ats_aug[:DIM, :] = A.T * s[None, :]
    ats_aug[DIM, :] = s
    ats_host = np.zeros((128, NKA, M), dtype=np.float32)
    ats_host[:, :NKC, :] = ats_aug.reshape(NKC, 128, M).transpose(1, 0, 2)
    ats_host[:M, NKC, :] = np.eye(M, dtype=np.float32)
    ats_host = np.ascontiguousarray(ats_host).astype(ndt)

    in_maps = []
    for c in range(N_CORES):
        c0, c1 = c * SH, (c + 1) * SH
        # wt1: W_layout^T columns chunk + bias row -> (128, 25, 384)
        w1t = np.zeros((KP, SH), dtype=np.float32)
        w1t[:DIM, :] = W_layout[c0:c1, :].T
        w1t[DIM, :] = b_layout[c0:c1]
        wt1_host = np.ascontiguousarray(
            w1t.reshape(NKC, 128, SH).transpose(1, 0, 2)).astype(ndt)
        # wt2: W_out^T rows chunk -> (128, 6, 3, 512)
        w2t = np.ascontiguousarray(W_out[:, c0:c1].T)    # (384, 3072)
        wt2_host = np.ascontiguousarray(
            w2t.reshape(NT, 128, NB, 512).transpose(1, 2, 0, 3)).astype(ndt)
        in_maps.append({"wt1": wt1_host, "wt2": wt2_host, "ats": ats_host})

    global _last_in_maps
    _last_in_maps = in_maps
    if "nc" not in _cache:
        _cache["nc"] = _build()
    res = run_bass_kernel_spmd(_cache["nc"], in_maps,
                               core_ids=list(range(N_CORES)))

    out16 = np.zeros((M, DIM), dtype=np.float32)
    for c in range(N_CORES):
        out16 += res.results[c]["out_partial"]

    # broadcast each ref row over its T tokens, then scatter by img_idxs
    T = img_idxs.shape[1]
    out_flat = np.repeat(
        out16.reshape(B, R, 1, DIM), T, axis=2).reshape(B, R * T, DIM)
    flat_idx = np.asarray(img_idxs, dtype=np.int64).reshape(-1)
    if R * T == N_IMG and np.array_equal(flat_idx, np.arange(N_IMG)):
        return np.ascontiguousarray(out_flat)
    result = image_tokens.copy()
    result[:, flat_idx, :] = out_flat
    return result


# revision 11
# speedup vs baseline: 1.7063x; 1.0375x over previous
"""Trainium2 Bass kernel for nn_AssembleAttentionBlock.

The reference module computes, per (batch b, ref r):
    kv[b,r,:]  = (instance_tokens[b,r] @ W_layout.T + b_layout) * (1 - alpha[b,r])
    scores     = q . kv   (per head)  -> softmax over a SINGLE key axis == 1.0
    attn       = kv (broadcast over the ref's T tokens, independent of q)
    out[b,r,:] = kv[b,r,:] @ W_out.T
    result     = image_tokens with rows img_idxs[r, :] replaced by out[b,r,:]

Since the softmax is over an axis of size 1, the whole query path
(gather + W_q projection + scores) cancels out, and each of the T tokens
of ref r receives the same row out[b,r,:].  The device kernel therefore
computes the two (16 x 3072) @ (3072 x 3072) projections, tensor-parallel
over the 3072 hidden dim across 8 NeuronCores:

  core c:  kv_c = scaled/biased A @ W_layout.T columns [c*384, (c+1)*384)
           partial_c = kv_c @ W_out.T rows [c*384, (c+1)*384)   (16 x 3072)
  host:    out16 = sum_c partial_c ; broadcast + scatter into image_tokens.

The bias row and the (1-alpha) scaling are folded into the matmul by
augmenting the contraction dim with one extra row (K = 3072+1 -> 3200 padded).
Weights are pre-arranged on host into the exact SBUF tile layouts so every
DMA is a fully contiguous block.  Matmuls keep the stationary operand tiny
(16 cols) and the moving operand large (384/512 cols); kv is transposed
on the PE (via a 16x16 identity) to feed stage 2's contraction.
"""

import os

import numpy as np

DIM = 3072
B = 2
R = 8
M = B * R            # 16 rows through both projections
N_IMG = 4096
N_CORES = 8
SH = DIM // N_CORES  # 384 hidden columns per core
NKC = 25             # contraction chunks of 128 (3072 + 1 bias row, zero-padded)
KP = NKC * 128       # 3200
NT = SH // 128       # 3 row-tiles of kv^T per core
NB = DIM // 512      # 6 output column banks
W1SPLIT = (9, 8, 4, 4)  # wt1 DMA chunks (k-blocks); small tail chunks
W2SPLIT = (3, 2, 1)     # wt2 DMA chunks (output banks); small tail chunk
NKA = NKC + 1        # ats chunks + identity chunk

MM_DTYPE = os.environ.get("BASS_MM_DTYPE", "float32r")

_cache = {}
_last_in_maps = None


def _np_in_dtype():
    if MM_DTYPE == "bfloat16":
        import ml_dtypes
        return ml_dtypes.bfloat16
    if MM_DTYPE == "float16":
        return np.float16
    return np.float32


def _build():
    """Build + compile the per-core Bass program (same NEFF on all cores).

    Raw-Bass (no TileContext): a small static program with hand-placed
    semaphores, to avoid Tile's ~300 event-semaphore instructions, its
    instruction-fetch startup cost and its end-of-kernel drain+barrier tail.

    Semaphore protocol:
      dmaI: +16 per input DMA, issue order ats, ident, wt1[0..4], wt2[0..2]
      pe:   1 = stage-1 accumulation done; 2 = transposes done;
            3+nb = stage-2 bank nb matmuls done
      dve:  1 = kv copied to SBUF; 2 = kv^T tiles copied;
            3+nb = output bank nb copied to SBUF
      dmaO: +16 per output DMA (6 total, issued on the ACT HWDGE ring so
            they never queue behind input DMAs on the SP ring)
    """
    from contextlib import ExitStack

    import concourse.bacc as bacc
    import concourse.mybir as mybir

    nc = bacc.Bacc("TRN2", target_bir_lowering=False, debug=False,
                   num_devices=N_CORES)
    DT = {
        "float32": mybir.dt.float32,
        "float32r": mybir.dt.float32r,
        "bfloat16": mybir.dt.bfloat16,
        "float16": mybir.dt.float16,
    }[MM_DTYPE]
    F32 = mybir.dt.float32

    wt1 = nc.dram_tensor("wt1", [128, NKC, SH], DT, kind="ExternalInput")
    wt2 = nc.dram_tensor("wt2", [128, NB, NT, 512], DT, kind="ExternalInput")
    ats = nc.dram_tensor("ats", [128, NKA, M], DT, kind="ExternalInput")
    outp = nc.dram_tensor("out_partial", [M, DIM], F32, kind="ExternalOutput")

    with ExitStack() as ctx:
        ats_sb = ctx.enter_context(nc.sbuf_tensor("ats_sb", [128, NKA, M], DT))
        wt1_sb = ctx.enter_context(nc.sbuf_tensor("wt1_sb", [128, NKC, SH], DT))
        wt2_sb = ctx.enter_context(
            nc.sbuf_tensor("wt2_sb", [128, NB, NT, 512], DT))
        kv_sb = ctx.enter_context(nc.sbuf_tensor("kv_sb", [M, SH], DT))
        kvT_sb = [ctx.enter_context(nc.sbuf_tensor(f"kvT_sb{t}", [128, M], DT))
                  for t in range(NT)]
        o_sb = [ctx.enter_context(nc.sbuf_tensor(f"o_sb{i}", [M, 512], F32))
                for i in range(NB)]

        kv_ps = ctx.enter_context(nc.psum_tensor("kv_ps", [M, SH], F32))
        kvT_ps = [ctx.enter_context(nc.psum_tensor(f"kvT_ps{t}", [128, M], DT))
                  for t in range(NT)]
        o_ps = [ctx.enter_context(nc.psum_tensor(f"o_ps{i}", [M, 512], F32))
                for i in range(2)]

        s_ats = ctx.enter_context(nc.semaphore("s_ats"))
        s_w1 = [ctx.enter_context(nc.semaphore(f"s_w1_{i}"))
                for i in range(len(W1SPLIT))]
        s_w2 = [ctx.enter_context(nc.semaphore(f"s_w2_{i}"))
                for i in range(len(W2SPLIT))]
        dmaO = ctx.enter_context(nc.semaphore("dmaO"))
        pe = ctx.enter_context(nc.semaphore("pe"))
        dve = ctx.enter_context(nc.semaphore("dve"))

        block = ctx.enter_context(nc.Block(no_gpsimd_drain=True))

        id_sb = ats_sb[:M, NKC, :]          # identity lives in ats chunk 25

        @block.sync
        def _(sync):
            sync.dma_start(ats_sb[:], ats[:]).then_inc(s_ats, 16)
            k0 = 0
            for i, n in enumerate(W1SPLIT):
                sync.dma_start(
                    wt1_sb[:, k0:k0 + n, :], wt1[:, k0:k0 + n, :]
                ).then_inc(s_w1[i], 16)
                k0 += n
            b0 = 0
            for i, n in enumerate(W2SPLIT):
                sync.dma_start(
                    wt2_sb[:, b0:b0 + n, :, :], wt2[:, b0:b0 + n, :, :]
                ).then_inc(s_w2[i], 16)
                b0 += n

        @block.tensor
        def _(tensor):
            # stage 1: kv (16 x 384) accumulated over 25 K-chunks
            k_edges = list(np.cumsum((0,) + W1SPLIT))[:-1]
            tensor.wait_ge(s_ats, 16)
            for k in range(NKC):
                if k in k_edges:
                    tensor.wait_ge(s_w1[k_edges.index(k)], 16)
                mm = tensor.matmul(
                    kv_ps[:],
                    ats_sb[:, k, :],
                    wt1_sb[:, k, :],
                    start=(k == 0),
                    stop=(k == NKC - 1),
                )
                if k == NKC - 1:
                    mm.then_inc(pe, 1)
            # transpose kv -> kv^T (3 x [128, 16])
            tensor.wait_ge(dve, 1)
            for t in range(NT):
                mm = tensor.transpose(
                    kvT_ps[t][:], kv_sb[:, 128 * t:128 * (t + 1)], id_sb)
                if t == NT - 1:
                    mm.then_inc(pe, 1)
            # stage 2: partial (16 x 3072), contraction over this core's 384
            tensor.wait_ge(dve, 2)
            nb_edges = list(np.cumsum((0,) + W2SPLIT))[:-1]
            for nb in range(NB):
                if nb in nb_edges:
                    tensor.wait_ge(s_w2[nb_edges.index(nb)], 16)
                if nb >= 2:
                    tensor.wait_ge(dve, 1 + nb)  # WAR: o_ps[nb%2] free again
                for j in range(NT):
                    mm = tensor.matmul(
                        o_ps[nb % 2][:],
                        kvT_sb[j][:],
                        wt2_sb[:, nb, j, :],
                        start=(j == 0),
                        stop=(j == NT - 1),
                    )
                    if j == NT - 1:
                        mm.then_inc(pe, 1)

        @block.vector
        def _(vector):
            vector.wait_ge(pe, 1)
            vector.tensor_copy(kv_sb[:], kv_ps[:]).then_inc(dve, 1)
            vector.wait_ge(pe, 2)
            for t in range(NT):
                cp = vector.tensor_copy(kvT_sb[t][:], kvT_ps[t][:])
                if t == NT - 1:
                    cp.then_inc(dve, 1)
            for nb in range(NB):
                vector.wait_ge(pe, 3 + nb)
                vector.tensor_copy(
                    o_sb[nb][:], o_ps[nb % 2][:]).then_inc(dve, 1)

        @block.scalar
        def _(scalar):
            for nb in range(NB):
                scalar.wait_ge(dve, 3 + nb)
                scalar.dma_start(
                    outp[:, 512 * nb:512 * (nb + 1)], o_sb[nb][:]
                ).then_inc(dmaO, 16)
            scalar.wait_ge(dmaO, 16 * NB)

    nc.compile()
    return nc


def kernel(instance_tokens, image_tokens, bbox_masks, alpha,
           W_layout, b_layout, W_q, W_out, img_idxs):
    from concourse.bass_utils import run_bass_kernel_spmd

    instance_tokens = np.asarray(instance_tokens, dtype=np.float32)
    image_tokens = np.asarray(image_tokens, dtype=np.float32)
    alpha = np.asarray(alpha, dtype=np.float32)
    W_layout = np.asarray(W_layout, dtype=np.float32)
    b_layout = np.asarray(b_layout, dtype=np.float32)
    W_out = np.asarray(W_out, dtype=np.float32)
    img_idxs = np.asarray(img_idxs)

    ndt = _np_in_dtype()
    A = instance_tokens.reshape(M, DIM)                  # (16, 3072)
    s = (1.0 - alpha).reshape(M).astype(np.float32)      # (16,)

    # ats: augmented, pre-scaled A^T -> SBUF layout (128, 25, 16)
    ats_aug = np.zeros((KP, M), dtype=np.float32)
    ats_aug[:DIM, :] = A.T * s[None, :]
    ats_aug[DIM, :] = s
    ats_host = np.zeros((128, NKA, M), dtype=np.float32)
    ats_host[:, :NKC, :] = ats_aug.reshape(NKC, 128, M).transpose(1, 0, 2)
    ats_host[:M, NKC, :] = np.eye(M, dtype=np.float32)
    ats_host = np.ascontiguousarray(ats_host).astype(ndt)

    in_maps = []
    for c in range(N_CORES):
        c0, c1 = c * SH, (c + 1) * SH
        # wt1: W_layout^T columns chunk + bias row -> (128, 25, 384)
        w1t = np.zeros((KP, SH), dtype=np.float32)
        w1t[:DIM, :] = W_layout[c0:c1, :].T
        w1t[DIM, :] = b_layout[c0:c1]
        wt1_host = np.ascontiguousarray(
            w1t.reshape(NKC, 128, SH).transpose(1, 0, 2)).astype(ndt)
        # wt2: W_out^T rows chunk -> (128, 6, 3, 512)
        w2t = np.ascontiguousarray(W_out[:, c0:c1].T)    # (384, 3072)
        wt2_host = np.ascontiguousarray(
            w2t.reshape(NT, 128, NB, 512).transpose(1, 2, 0, 3)).astype(ndt)
        in_maps.append({"wt1": wt1_host, "wt2": wt2_host, "ats": ats_host})

    global _last_in_maps
    _last_in_maps = in_maps
    if "nc" not in _cache:
        _cache["nc"] = _build()
    res = run_bass_kernel_spmd(_cache["nc"], in_maps,
                               core_ids=list(range(N_CORES)))

    out16 = np.zeros((M, DIM), dtype=np.float32)
    for c in range(N_CORES):
        out16 += res.results[c]["out_partial"]

    # broadcast each ref row over its T tokens, then scatter by img_idxs
    T = img_idxs.shape[1]
    out_flat = np.repeat(
        out16.reshape(B, R, 1, DIM), T, axis=2).reshape(B, R * T, DIM)
    flat_idx = np.asarray(img_idxs, dtype=np.int64).reshape(-1)
    if R * T == N_IMG and np.array_equal(flat_idx, np.arange(N_IMG)):
        return np.ascontiguousarray(out_flat)
    result = image_tokens.copy()
    result[:, flat_idx, :] = out_flat
    return result
